# revision 62
# baseline (speedup 1.0000x reference)
"""Multi-head attention (B=2, N=2048, D=1024, H=16, RoPE, dense softmax) on
8 Trainium2 NeuronCores.

Sharding: data-parallel over batch (cores 0-3 -> b=0, 4-7 -> b=1), tensor-
parallel over heads (each core owns 4 of the 16 heads, i.e. 256 of the 1024
hidden dims of Wq/Wk/Wv rows and Wo columns). Each core computes its heads'
attention and a partial output projection; the host sums the 4 partials per
batch.

Single merged pipeline: the attention chunk loop starts as soon as K/Q for
head-pair 0 and the first V chunks exist; all remaining projection work
(RoPE'd Q/K tiles, V chunks) plus the output projection is chopped into
small "filler" thunks that are drained between attention chunk iterations,
keeping the PE and DVE busy underneath the ACT-bound exp stream.

Key device-level choices:
  - matmul operands fp16 (1 cyc/row + fast weight load); PSUM fp32.
  - scores computed as S^T [keys, q] in double-buffered 2-bank PSUM tiles;
    exp on ScalarE in [128,1024] tiles (the pipeline's rate limiter).
  - V carries a leading ones column so the P@V matmul also emits the softmax
    denominators in PSUM partition 0 (readable by the custom fast-reciprocal,
    which cannot read PSUM at partition offset 64).
  - per-iteration issue order is QK(next chunk) -> exp -> fillers -> PV so a
    lagging filler or PV never starves the ScalarE exp stream.
  - weights/x/tables are host-prepacked into the exact SBUF layouts so every
    input DMA is a contiguous [128, 1024+] transfer.
"""

import collections
import os
import numpy as np

import concourse.bass as bass
from concourse import bacc
import concourse.mybir as mybir
import concourse.tile as tile
from concourse.bass_utils import run_bass_kernel_spmd

dt = mybir.dt

B, N, D, H, HD = 2, 2048, 1024, 16, 64
NCORES = 8
HPC = H * B // NCORES          # 4 heads per core
DPC = HPC * HD                 # 256 owned hidden dims per core
QT = 512                       # query tile (free dim of QK^T / PV matmuls)
NQT = N // QT                  # 4 query tiles
KC = 128                       # key chunk (partition dim of S^T)
NKC = N // KC                  # 16 key chunks
DC = D // 128                  # 8 contraction chunks for projections
SCALE = float(HD) ** -0.5
# Global exp bias: scores reach ~7.3, exp(7.3)=1518 overflows fp8e4m3 (max
# 448). exp(s-3) tops out at ~76; the bias cancels exactly in the softmax
# because the ones-column denominator sums the same biased exps.
EXP_BIAS = -3.0

F16 = dt.float16
F32 = dt.float32
FP8 = dt.float8e4
NP16 = np.float16
VP = 68      # fp8 V row padded to 68 cols so the DoubleRow Ko stride (4*68B)
             # stays 16-byte aligned


def build_nc():
    nc = bacc.Bacc("TRN2")
    xT = nc.dram_tensor("xT", [D, N], F16, kind="ExternalInput")
    wqd = nc.dram_tensor("wqd", [128, DC * DPC], F16, kind="ExternalInput")
    wkd = nc.dram_tensor("wkd", [128, DC * DPC], F16, kind="ExternalInput")
    wvd = nc.dram_tensor("wvd", [128, DC * DPC], F16, kind="ExternalInput")
    wod = nc.dram_tensor("wod", [128, 2 * D], F16, kind="ExternalInput")
    cosd = nc.dram_tensor("cosd", [128, N], F16, kind="ExternalInput")
    # msin with partition rows pre-permuted by r^32 so the RoPE rotate-half
    # multiply reads both SBUF operands from the same base partition.
    msinPd = nc.dram_tensor("msinPd", [128, N], F16, kind="ExternalInput")
    out = nc.dram_tensor("out", [N, D], F16, kind="ExternalOutput")
    dbg = os.environ.get("KDEBUG")
    if dbg:
        dbg_qT = nc.dram_tensor("dbg_qT", [128, 2, N], F16, kind="ExternalOutput")
        dbg_kT = nc.dram_tensor("dbg_kT", [128, 2, N], F16, kind="ExternalOutput")
        dbg_v = nc.dram_tensor("dbg_v", [128, NKC, HPC, HD + 1], F16, kind="ExternalOutput")
        dbg_attnT = nc.dram_tensor("dbg_attnT", [128, 2, N], F16, kind="ExternalOutput")

    with tile.TileContext(nc) as tc:
        with tc.tile_pool(name="xp", bufs=1) as xp, \
             tc.tile_pool(name="persist", bufs=1) as persist, \
             tc.tile_pool(name="ptp", bufs=9) as ptp, \
             tc.tile_pool(name="ropep", bufs=2) as ropep, \
             tc.tile_pool(name="normp", bufs=2) as normp, \
             tc.tile_pool(name="outp", bufs=4) as outp, \
             tc.tile_pool(name="ps_st", bufs=2, space="PSUM") as ps_st, \
             tc.tile_pool(name="ps_acc", bufs=1, space="PSUM") as ps_acc, \
             tc.tile_pool(name="ps_misc", bufs=2, space="PSUM") as ps_misc:

            # ---- persistent SBUF tensors; DMAs in dependency-priority order
            wv_s = persist.tile([128, DC, DPC], F16, name="wv_s")
            wk_s = persist.tile([128, DC, DPC], F16, name="wk_s")
            wq_s = persist.tile([128, DC, DPC], F16, name="wq_s")
            nc.sync.dma_start(out=wk_s, in_=wkd[:, :])
            nc.sync.dma_start(out=wq_s, in_=wqd[:, :])
            x_s = []
            for d in range(DC):
                xt = xp.tile([128, N], F16, name=f"x_s{d}", tag="x", bufs=DC)
                # alternate DMA queues (SP / Activation HWDGE) to halve the
                # prologue x-load latency
                eng = nc.sync if d % 2 == 0 else nc.scalar
                eng.dma_start(out=xt[:, 0:1024],
                              in_=xT[d * 128:(d + 1) * 128, 0:1024])
                x_s.append(xt)
            cos_s = persist.tile([128, N], F16, name="cos_s")
            msinP_s = persist.tile([128, N], F16, name="msinP_s")
            nc.sync.dma_start(out=cos_s, in_=cosd[:, :])
            nc.scalar.dma_start(out=msinP_s, in_=msinPd[:, :])
            nc.sync.dma_start(out=wv_s, in_=wvd[:, :])
            for d in range(DC):
                eng = nc.sync if d % 2 == 0 else nc.scalar
                eng.dma_start(out=x_s[d][:, 1024:2048],
                              in_=xT[d * 128:(d + 1) * 128, 1024:2048])
            wo_s = persist.tile([128, 2, D], F16, name="wo_s")
            nc.sync.dma_start(out=wo_s, in_=wod[:, :])

            qT_s = persist.tile([128, 2, N], F16, name="qT_s")
            kT_s = persist.tile([128, 2, N], F16, name="kT_s")
            # V with trailing ones column: [keys(128), kchunk, head, 64+1].
            # The ones column makes the P@V matmul also emit softmax
            # denominators (ones last, not first: the PV output's O^T rows
            # must start at partition 0 — partition offsets are 32-aligned).
            v_s = persist.tile([128, NKC, HPC, HD + 1], F16, name="v_s")
            nc.gpsimd.memset(v_s[:, :, :, HD:HD + 1], 1.0)
            attnT_s = persist.tile([128, 2, N], F16, name="attnT_s")
            ebias_s = persist.tile([128, 1], F32, name="ebias_s")
            nc.gpsimd.memset(ebias_s, EXP_BIAS)

            # ---- projection / filler units (generators yielding ~400ns of
            # PE work per step so they can be drained between chunk iters)
            def gen_proj(w_s, dstT, i, t):
                """Project one [128, QT] q/k tile for head-pair i and RoPE it."""
                ps = ps_misc.tile([128, QT], F32, name="pps", tag="m")
                for d2 in range(DC // 2):
                    for d in (2 * d2, 2 * d2 + 1):
                        nc.tensor.matmul(
                            ps, w_s[:, d, i * 128:(i + 1) * 128],
                            x_s[d][:, t * QT:(t + 1) * QT],
                            start=(d == 0), stop=(d == DC - 1))
                    yield
                # stage PSUM->SBUF fp16 (frees the misc bank fast), then RoPE
                # entirely in fp16 at 2x DVE rate.
                qf = ropep.tile([128, QT], F16, name="qf", tag="qf")
                nc.vector.tensor_copy(out=qf, in_=ps)
                cs = cos_s[:, t * QT:(t + 1) * QT]
                ms = msinP_s[:, t * QT:(t + 1) * QT]
                tf = ropep.tile([128, QT], F16, name="tf", tag="tf")
                for r in (0, 32, 64, 96):
                    pr = r ^ 32
                    nc.vector.tensor_mul(out=tf[r:r + 32, :],
                                         in0=qf[pr:pr + 32, :],
                                         in1=ms[pr:pr + 32, :])
                dst = dstT[:, i, t * QT:(t + 1) * QT]
                nc.vector.tensor_mul(out=dst, in0=qf, in1=cs)
                nc.vector.tensor_add(out=dst, in0=dst, in1=tf)
                yield

            def gen_v(k):
                """Project V chunk k (all 4 heads) into v_s[:, k, :, 1:]."""
                pv = ps_misc.tile([128, DPC], F32, name="pvv", tag="m")
                for d2 in range(DC // 2):
                    for d in (2 * d2, 2 * d2 + 1):
                        nc.tensor.matmul(pv,
                                         x_s[d][:, k * KC:(k + 1) * KC],
                                         wv_s[:, d, :],
                                         start=(d == 0), stop=(d == DC - 1))
                    yield
                nc.vector.tensor_copy(
                    out=v_s[:, k, :, 0:HD],
                    in_=pv.rearrange("p (h e) -> p h e", h=HPC))
                yield

            def gen_outproj(t2q, qc):
                """Output projection for one 128-query chunk of tile-pair t2q."""
                q0 = t2q * 2 * QT + qc * 128
                ot = outp.tile([128, D], F16, name="ot", tag="ot")
                for e in range(2):
                    pos = ps_misc.tile([128, 512], F32, name="pos", tag="m")
                    for dc in range(2):
                        nc.tensor.matmul(
                            pos, attnT_s[:, dc, q0:q0 + 128],
                            wo_s[:, dc, e * 512:(e + 1) * 512],
                            start=(dc == 0), stop=(dc == 1))
                        yield
                    nc.vector.tensor_copy(out=ot[:, e * 512:(e + 1) * 512],
                                          in_=pos)
                nc.sync.dma_start(out=out[q0:q0 + 128, :], in_=ot)
                yield

            filler = collections.deque()
            done = set()

            def tracked(g, key):
                yield from g
                done.add(key)

            def drain(n):
                for _ in range(n):
                    while filler:
                        try:
                            next(filler[0])
                            break
                        except StopIteration:
                            filler.popleft()
                    else:
                        break

            def force(key):
                # Correctness guard: a consumer must never be ISSUED before
                # its producer (Tile tracks deps in issue order), so drain
                # the filler queue until the producer unit has been emitted.
                while key not in done:
                    assert filler, f"filler ran dry before {key}"
                    drain(1)

            def run_unit(g, key):
                for _ in g:
                    pass
                done.add(key)

            # ---- prologue: just enough for attention block 0 to start.
            # K0t0/Q0t0/Q0t1 first — their RoPEs gate the first exp.
            run_unit(gen_proj(wk_s, kT_s, 0, 0), ("k", 0, 0))
            run_unit(gen_proj(wq_s, qT_s, 0, 0), ("q", 0, 0))
            run_unit(gen_proj(wq_s, qT_s, 0, 1), ("q", 0, 1))
            run_unit(gen_v(0), ("v", 0))

            # filler queue ordered by when each unit's output is consumed
            # (K tiles need ~3 iters of RoPE margin before their chunks)
            units = [
                (("k", 0, 1), gen_proj(wk_s, kT_s, 0, 1)),
                (("v", 1), gen_v(1)),
                (("v", 2), gen_v(2)), (("v", 3), gen_v(3)),
                (("k", 0, 2), gen_proj(wk_s, kT_s, 0, 2)),
                (("v", 4), gen_v(4)),
                (("k", 0, 3), gen_proj(wk_s, kT_s, 0, 3)),
                (("v", 5), gen_v(5)), (("v", 6), gen_v(6)),
                (("v", 7), gen_v(7)), (("v", 8), gen_v(8)),
                (("v", 9), gen_v(9)), (("v", 10), gen_v(10)),
                (("v", 11), gen_v(11)), (("v", 12), gen_v(12)),
                (("v", 13), gen_v(13)), (("v", 14), gen_v(14)),
                (("v", 15), gen_v(15)),
                (("q", 0, 2), gen_proj(wq_s, qT_s, 0, 2)),
                (("q", 0, 3), gen_proj(wq_s, qT_s, 0, 3)),
                (("k", 1, 0), gen_proj(wk_s, kT_s, 1, 0)),
                (("q", 1, 0), gen_proj(wq_s, qT_s, 1, 0)),
                (("q", 1, 1), gen_proj(wq_s, qT_s, 1, 1)),
                (("k", 1, 1), gen_proj(wk_s, kT_s, 1, 1)),
                (("k", 1, 2), gen_proj(wk_s, kT_s, 1, 2)),
                (("k", 1, 3), gen_proj(wk_s, kT_s, 1, 3)),
                (("q", 1, 2), gen_proj(wq_s, qT_s, 1, 2)),
                (("q", 1, 3), gen_proj(wq_s, qT_s, 1, 3)),
            ]
            for key, g in units:
                filler.append(tracked(g, key))

            # ---- attention: 8 blocks x 16 key chunks, issued as ONE
            # continuous stream. PV + the block normalize trail the QK/exp
            # stream by TRAIL chunks so a block boundary never halts the
            # ScalarE exp pipeline (a halt idles the PE >3.4us and the HAM
            # clock-gate re-throttles it to half rate).
            TRAIL = 6

            def norm_block(acc, r0, i, t2q, t2q_done):
                # Release the single acc PSUM buffer ASAP: two fast bulk
                # copies stage the denominators (f32) and the unnormalized
                # O^T rows (f16) to SBUF; the reciprocal/broadcast/multiply
                # then run off-critical-path on the SBUF copies. (The custom
                # DVE reciprocal can't read PSUM at partition 64, so den is
                # staged anyway.)
                den_raw = normp.tile([1, 2, QT], F32, name="den_raw",
                                     tag="denr")
                nc.vector.tensor_copy(out=den_raw, in_=acc[HD:HD + 1, :, :])
                o16 = normp.tile([HD, 2, QT], F16, name="o16", tag="o16")
                nc.vector.tensor_copy(out=o16, in_=acc[0:HD, :, :])
                for u in range(2):
                    den = normp.tile([1, QT], F32, name="den", tag="den")
                    nc.vector.reciprocal_approx_fast(out=den,
                                                     in_=den_raw[:, u, :])
                    bca = normp.tile([HD, QT], F32, name="bca", tag="bca")
                    nc.gpsimd.partition_broadcast(bca, den)
                    t = 2 * t2q + u
                    nc.vector.tensor_mul(
                        out=attnT_s[r0:r0 + HD, i, t * QT:(t + 1) * QT],
                        in0=o16[:, u, :], in1=bca)
                if t2q_done:
                    # this tile-pair's attnT is complete -> its output
                    # projection becomes legal filler work
                    for qc in range(2 * QT // 128):
                        filler.append(gen_outproj(t2q, qc))

            def pv_of(k, pt, acc, h, r0, i, t2q):
                force(("v", k))
                vsl = v_s[:, k, h, :]
                for u in range(2):
                    nc.tensor.matmul(
                        acc[:, u, :], vsl, pt[:, u, :],
                        start=(k == 0), stop=(k == NKC - 1),
                        skip_group_check=True)

            pend = collections.deque()

            def flush_one():
                k, pt, acc, h, r0, i, t2q = pend.popleft()
                pv_of(k, pt, acc, h, r0, i, t2q)
                if k == NKC - 1:
                    norm_block(acc, r0, i, t2q,
                               t2q_done=(i == 1 and r0 == HD))

            # blocks ordered i-outer: head-pair 1's projections aren't
            # needed until iteration 64, spreading filler work into the
            # later blocks' PE slack
            rates = [3, 3, 2, 2, 2, 2, 1, 1]
            for i in range(2):
                for t2q in range(2):
                    for hl in range(2):
                        rate = rates[i * 4 + t2q * 2 + hl]
                        h = i * 2 + hl
                        r0 = hl * HD
                        force(("q", i, 2 * t2q))
                        force(("q", i, 2 * t2q + 1))
                        acc = ps_acc.tile([HD + 1, 2, QT], F32, name="acc",
                                          tag="acc")
                        qsl = [qT_s[r0:r0 + HD, i,
                                    (2 * t2q + u) * QT:(2 * t2q + u + 1) * QT]
                               for u in range(2)]
                        for k in range(NKC):
                            if k % 4 == 0:
                                force(("k", i, k // 4))
                            st = ps_st.tile([128, 2, QT], F32, name="st",
                                            tag="st")
                            ksl = kT_s[r0:r0 + HD, i, k * KC:(k + 1) * KC]
                            for u in range(2):
                                nc.tensor.matmul(st[:, u, :], ksl, qsl[u],
                                                 start=True, stop=True)
                            pt = ptp.tile([128, 2, QT], F16, name="pt",
                                          tag="pt")
                            nc.scalar.activation(
                                out=pt.rearrange("p a b -> p (a b)"),
                                in_=st.rearrange("p a b -> p (a b)"),
                                func=mybir.ActivationFunctionType.Exp,
                                bias=ebias_s, scale=SCALE)
                            drain(rate)
                            pend.append((k, pt, acc, h, r0, i, t2q))
                            if len(pend) > TRAIL:
                                flush_one()
            while pend:
                flush_one()
                drain(2)
            drain(10 ** 9)
            if dbg:
                nc.sync.dma_start(out=dbg_qT[:, :, :], in_=qT_s)
                nc.sync.dma_start(out=dbg_kT[:, :, :], in_=kT_s)
                nc.sync.dma_start(out=dbg_v[:, :, :, :], in_=v_s)
                nc.sync.dma_start(out=dbg_attnT[:, :, :], in_=attnT_s)
    nc.finalize()
    return nc


_NC_CACHE = None


def _get_nc():
    global _NC_CACHE
    if _NC_CACHE is None:
        _NC_CACHE = build_nc()
    return _NC_CACHE


def _rope_tables():
    inv_freq = 1.0 / (10000.0 ** (np.arange(0, HD, 2, dtype=np.float32) / HD))
    t = np.arange(N, dtype=np.float32)
    freqs = np.outer(t, inv_freq).astype(np.float32)       # [N, 32]
    emb = np.concatenate([freqs, freqs], axis=-1)          # [N, 64]
    cos = np.cos(emb).astype(np.float32)                   # [N, 64]
    sin = np.sin(emb).astype(np.float32)
    idx = np.arange(128) % HD
    cosT = np.ascontiguousarray(cos.T[idx])                # [128, N]
    sgn = np.where(np.arange(HD) < HD // 2, -1.0, 1.0).astype(np.float32)
    msinT = np.ascontiguousarray((sin.T * sgn[:, None])[idx])
    msinP = np.ascontiguousarray(msinT[np.arange(128) ^ 32])
    return cosT.astype(NP16), msinP.astype(NP16)


def _pack_w(wT):
    """[n*128, C] row-major -> [128, n*C] with [p, chunk, c] free layout."""
    n = wT.shape[0] // 128
    return np.ascontiguousarray(
        wT.reshape(n, 128, -1).transpose(1, 0, 2).reshape(128, -1)
    ).astype(NP16)


def kernel(x, attention_mask, Wq, Wk, Wv, Wo):
    x = np.asarray(x, dtype=np.float32)
    Wq = np.asarray(Wq, dtype=np.float32)
    Wk = np.asarray(Wk, dtype=np.float32)
    Wv = np.asarray(Wv, dtype=np.float32)
    Wo = np.asarray(Wo, dtype=np.float32)

    cosT, msinP = _rope_tables()
    xTb = [np.ascontiguousarray(x[b].T).astype(NP16) for b in range(B)]

    in_maps = []
    for c in range(NCORES):
        b = c // (NCORES // B)
        hg = c % (NCORES // B)
        rows = slice(hg * DPC, (hg + 1) * DPC)
        in_maps.append({
            "xT": xTb[b],
            "wqd": _pack_w(Wq[rows].T),
            "wkd": _pack_w(Wk[rows].T),
            "wvd": _pack_w(Wv[rows].T),
            "wod": _pack_w(Wo[:, rows].T),
            "cosd": cosT,
            "msinPd": msinP,
        })

    global _last_in_maps
    _last_in_maps = in_maps

    nc = _get_nc()
    res = run_bass_kernel_spmd(nc, in_maps, core_ids=list(range(NCORES)))
    global _LAST_RES
    _LAST_RES = res
    parts = [r["out"].astype(np.float32) for r in res.results]

    out = np.empty((B, N, D), dtype=np.float32)
    g = NCORES // B
    for b in range(B):
        out[b] = np.sum(np.stack(parts[b * g:(b + 1) * g]), axis=0)
    return out


# revision 63
# speedup vs baseline: 1.1797x; 1.1797x over previous
"""Multi-head attention (B=2, N=2048, D=1024, H=16, RoPE, dense softmax) on
8 Trainium2 NeuronCores.

Sharding: data-parallel over batch (cores 0-3 -> b=0, 4-7 -> b=1), tensor-
parallel over heads (each core owns 4 of the 16 heads, i.e. 256 of the 1024
hidden dims of Wq/Wk/Wv rows and Wo columns). Each core computes its heads'
attention and a partial output projection; the host sums the 4 partials per
batch.

Single merged pipeline: the attention chunk loop starts as soon as K/Q for
head-pair 0 and the first V chunks exist; all remaining projection work
(RoPE'd Q/K tiles, V chunks) plus the output projection is chopped into
small "filler" thunks that are drained between attention chunk iterations,
keeping the PE and DVE busy underneath the ACT-bound exp stream.

Key device-level choices:
  - matmul operands fp16 (1 cyc/row + fast weight load); PSUM fp32.
  - scores computed as S^T [keys, q] in double-buffered 2-bank PSUM tiles;
    exp on ScalarE in [128,1024] tiles (the pipeline's rate limiter).
  - V carries a leading ones column so the P@V matmul also emits the softmax
    denominators in PSUM partition 0 (readable by the custom fast-reciprocal,
    which cannot read PSUM at partition offset 64).
  - per-iteration issue order is QK(next chunk) -> exp -> fillers -> PV so a
    lagging filler or PV never starves the ScalarE exp stream.
  - weights/x/tables are host-prepacked into the exact SBUF layouts so every
    input DMA is a contiguous [128, 1024+] transfer.
"""

import collections
import os
import numpy as np

import concourse.bass as bass
from concourse import bacc
import concourse.mybir as mybir
import concourse.tile as tile
from concourse.bass_utils import run_bass_kernel_spmd

dt = mybir.dt

B, N, D, H, HD = 2, 2048, 1024, 16, 64
NCORES = 8
HPC = H * B // NCORES          # 4 heads per core
DPC = HPC * HD                 # 256 owned hidden dims per core
QT = 512                       # query tile (free dim of QK^T / PV matmuls)
NQT = N // QT                  # 4 query tiles
KC = 128                       # key chunk (partition dim of S^T)
NKC = N // KC                  # 16 key chunks
DC = D // 128                  # 8 contraction chunks for projections
SCALE = float(HD) ** -0.5
# Global exp bias: scores reach ~7.3, exp(7.3)=1518 overflows fp8e4m3 (max
# 448). exp(s-3) tops out at ~76; the bias cancels exactly in the softmax
# because the ones-column denominator sums the same biased exps.
EXP_BIAS = -3.0

F16 = dt.float16
F32 = dt.float32
FP8 = dt.float8e4
NP16 = np.float16
VP = 68      # fp8 V row padded to 68 cols so the DoubleRow Ko stride (4*68B)
             # stays 16-byte aligned


def build_nc():
    nc = bacc.Bacc("TRN2")
    xT = nc.dram_tensor("xT", [D, N], F16, kind="ExternalInput")
    wqd = nc.dram_tensor("wqd", [128, DC * DPC], F16, kind="ExternalInput")
    wkd = nc.dram_tensor("wkd", [128, DC * DPC], F16, kind="ExternalInput")
    wvd = nc.dram_tensor("wvd", [128, DC * DPC], F16, kind="ExternalInput")
    wod = nc.dram_tensor("wod", [128, 2 * D], F16, kind="ExternalInput")
    cosd = nc.dram_tensor("cosd", [128, N], F16, kind="ExternalInput")
    # msin with partition rows pre-permuted by r^32 so the RoPE rotate-half
    # multiply reads both SBUF operands from the same base partition.
    msinPd = nc.dram_tensor("msinPd", [128, N], F16, kind="ExternalInput")
    out = nc.dram_tensor("out", [N, D], F16, kind="ExternalOutput")
    dbg = os.environ.get("KDEBUG")
    if dbg:
        dbg_qT = nc.dram_tensor("dbg_qT", [128, 2, N], F16, kind="ExternalOutput")
        dbg_kT = nc.dram_tensor("dbg_kT", [128, 2, N], F16, kind="ExternalOutput")
        dbg_v = nc.dram_tensor("dbg_v", [128, NKC, HPC, HD + 1], F16, kind="ExternalOutput")
        dbg_attnT = nc.dram_tensor("dbg_attnT", [128, 2, N], F16, kind="ExternalOutput")

    with tile.TileContext(nc) as tc:
        with tc.tile_pool(name="xp", bufs=1) as xp, \
             tc.tile_pool(name="persist", bufs=1) as persist, \
             tc.tile_pool(name="ptp", bufs=9) as ptp, \
             tc.tile_pool(name="ropep", bufs=2) as ropep, \
             tc.tile_pool(name="normp", bufs=2) as normp, \
             tc.tile_pool(name="outp", bufs=4) as outp, \
             tc.tile_pool(name="ps_st", bufs=2, space="PSUM") as ps_st, \
             tc.tile_pool(name="ps_acc", bufs=1, space="PSUM") as ps_acc, \
             tc.tile_pool(name="ps_misc", bufs=2, space="PSUM") as ps_misc:

            # ---- persistent SBUF tensors; DMAs in dependency-priority order
            wv_s = persist.tile([128, DC, DPC], F16, name="wv_s")
            wk_s = persist.tile([128, DC, DPC], F16, name="wk_s")
            wq_s = persist.tile([128, DC, DPC], F16, name="wq_s")
            nc.sync.dma_start(out=wk_s, in_=wkd[:, :])
            nc.sync.dma_start(out=wq_s, in_=wqd[:, :])
            x_s = []
            for d in range(DC):
                xt = xp.tile([128, N], F16, name=f"x_s{d}", tag="x", bufs=DC)
                # alternate DMA queues (SP / Activation HWDGE) to halve the
                # prologue x-load latency
                eng = nc.sync if d % 2 == 0 else nc.scalar
                eng.dma_start(out=xt[:, 0:1024],
                              in_=xT[d * 128:(d + 1) * 128, 0:1024])
                x_s.append(xt)
            cos_s = persist.tile([128, N], F16, name="cos_s")
            msinP_s = persist.tile([128, N], F16, name="msinP_s")
            nc.sync.dma_start(out=cos_s, in_=cosd[:, :])
            nc.scalar.dma_start(out=msinP_s, in_=msinPd[:, :])
            nc.sync.dma_start(out=wv_s, in_=wvd[:, :])
            for d in range(DC):
                eng = nc.sync if d % 2 == 0 else nc.scalar
                eng.dma_start(out=x_s[d][:, 1024:2048],
                              in_=xT[d * 128:(d + 1) * 128, 1024:2048])
            wo_s = persist.tile([128, 2, D], F16, name="wo_s")
            nc.sync.dma_start(out=wo_s, in_=wod[:, :])

            qT_s = persist.tile([128, 2, N], F16, name="qT_s")
            kT_s = persist.tile([128, 2, N], F16, name="kT_s")
            # V with trailing ones column: [keys(128), kchunk, head, 64+1].
            # The ones column makes the P@V matmul also emit softmax
            # denominators (ones last, not first: the PV output's O^T rows
            # must start at partition 0 — partition offsets are 32-aligned).
            v_s = persist.tile([128, NKC, HPC, HD + 1], F16, name="v_s")
            nc.gpsimd.memset(v_s[:, :, :, HD:HD + 1], 1.0)
            attnT_s = persist.tile([128, 2, N], F16, name="attnT_s")
            ebias_s = persist.tile([128, 1], F32, name="ebias_s")
            nc.gpsimd.memset(ebias_s, EXP_BIAS)

            # ---- projection / filler units (generators yielding ~400ns of
            # PE work per step so they can be drained between chunk iters)
            def gen_proj(w_s, dstT, i, t):
                """Project one [128, QT] q/k tile for head-pair i and RoPE it."""
                ps = ps_misc.tile([128, QT], F32, name="pps", tag="m")
                for d2 in range(DC // 2):
                    for d in (2 * d2, 2 * d2 + 1):
                        nc.tensor.matmul(
                            ps, w_s[:, d, i * 128:(i + 1) * 128],
                            x_s[d][:, t * QT:(t + 1) * QT],
                            start=(d == 0), stop=(d == DC - 1))
                    yield
                # stage PSUM->SBUF fp16 (frees the misc bank fast), then RoPE
                # entirely in fp16 at 2x DVE rate.
                qf = ropep.tile([128, QT], F16, name="qf", tag="qf")
                nc.vector.tensor_copy(out=qf, in_=ps)
                cs = cos_s[:, t * QT:(t + 1) * QT]
                ms = msinP_s[:, t * QT:(t + 1) * QT]
                tf = ropep.tile([128, QT], F16, name="tf", tag="tf")
                for r in (0, 32, 64, 96):
                    pr = r ^ 32
                    nc.vector.tensor_mul(out=tf[r:r + 32, :],
                                         in0=qf[pr:pr + 32, :],
                                         in1=ms[pr:pr + 32, :])
                dst = dstT[:, i, t * QT:(t + 1) * QT]
                nc.vector.tensor_mul(out=dst, in0=qf, in1=cs)
                nc.vector.tensor_add(out=dst, in0=dst, in1=tf)
                yield

            def gen_v(k):
                """Project V chunk k (all 4 heads) into v_s[:, k, :, 1:]."""
                pv = ps_misc.tile([128, DPC], F32, name="pvv", tag="m")
                for d2 in range(DC // 2):
                    for d in (2 * d2, 2 * d2 + 1):
                        nc.tensor.matmul(pv,
                                         x_s[d][:, k * KC:(k + 1) * KC],
                                         wv_s[:, d, :],
                                         start=(d == 0), stop=(d == DC - 1))
                    yield
                nc.vector.tensor_copy(
                    out=v_s[:, k, :, 0:HD],
                    in_=pv.rearrange("p (h e) -> p h e", h=HPC))
                yield

            def gen_outproj(t2q, qc):
                """Output projection for one 128-query chunk of tile-pair t2q."""
                q0 = t2q * 2 * QT + qc * 128
                ot = outp.tile([128, D], F16, name="ot", tag="ot")
                for e in range(2):
                    pos = ps_misc.tile([128, 512], F32, name="pos", tag="m")
                    for dc in range(2):
                        nc.tensor.matmul(
                            pos, attnT_s[:, dc, q0:q0 + 128],
                            wo_s[:, dc, e * 512:(e + 1) * 512],
                            start=(dc == 0), stop=(dc == 1))
                        yield
                    nc.vector.tensor_copy(out=ot[:, e * 512:(e + 1) * 512],
                                          in_=pos)
                nc.gpsimd.dma_start(out=out[q0:q0 + 128, :], in_=ot)
                yield

            filler = collections.deque()
            done = set()

            def tracked(g, key):
                yield from g
                done.add(key)

            def drain(n):
                for _ in range(n):
                    while filler:
                        try:
                            next(filler[0])
                            break
                        except StopIteration:
                            filler.popleft()
                    else:
                        break

            def force(key):
                # Correctness guard: a consumer must never be ISSUED before
                # its producer (Tile tracks deps in issue order), so drain
                # the filler queue until the producer unit has been emitted.
                while key not in done:
                    assert filler, f"filler ran dry before {key}"
                    drain(1)

            def run_unit(g, key):
                for _ in g:
                    pass
                done.add(key)

            # ---- prologue: just enough for attention block 0 to start.
            # K0t0/Q0t0/Q0t1 first — their RoPEs gate the first exp.
            run_unit(gen_proj(wk_s, kT_s, 0, 0), ("k", 0, 0))
            run_unit(gen_proj(wq_s, qT_s, 0, 0), ("q", 0, 0))
            run_unit(gen_proj(wq_s, qT_s, 0, 1), ("q", 0, 1))
            run_unit(gen_v(0), ("v", 0))

            # filler queue ordered by when each unit's output is consumed
            # (K tiles need ~3 iters of RoPE margin before their chunks)
            units = [
                (("k", 0, 1), gen_proj(wk_s, kT_s, 0, 1)),
                (("v", 1), gen_v(1)),
                (("v", 2), gen_v(2)), (("v", 3), gen_v(3)),
                (("k", 0, 2), gen_proj(wk_s, kT_s, 0, 2)),
                (("v", 4), gen_v(4)),
                (("k", 0, 3), gen_proj(wk_s, kT_s, 0, 3)),
                (("v", 5), gen_v(5)), (("v", 6), gen_v(6)),
                (("v", 7), gen_v(7)), (("v", 8), gen_v(8)),
                (("v", 9), gen_v(9)), (("v", 10), gen_v(10)),
                (("v", 11), gen_v(11)), (("v", 12), gen_v(12)),
                (("v", 13), gen_v(13)), (("v", 14), gen_v(14)),
                (("v", 15), gen_v(15)),
                (("q", 0, 2), gen_proj(wq_s, qT_s, 0, 2)),
                (("q", 0, 3), gen_proj(wq_s, qT_s, 0, 3)),
                (("k", 1, 0), gen_proj(wk_s, kT_s, 1, 0)),
                (("q", 1, 0), gen_proj(wq_s, qT_s, 1, 0)),
                (("q", 1, 1), gen_proj(wq_s, qT_s, 1, 1)),
                (("k", 1, 1), gen_proj(wk_s, kT_s, 1, 1)),
                (("k", 1, 2), gen_proj(wk_s, kT_s, 1, 2)),
                (("k", 1, 3), gen_proj(wk_s, kT_s, 1, 3)),
                (("q", 1, 2), gen_proj(wq_s, qT_s, 1, 2)),
                (("q", 1, 3), gen_proj(wq_s, qT_s, 1, 3)),
            ]
            for key, g in units:
                filler.append(tracked(g, key))

            # ---- attention: 8 blocks x 16 key chunks, issued as ONE
            # continuous stream. PV + the block normalize trail the QK/exp
            # stream by TRAIL chunks so a block boundary never halts the
            # ScalarE exp pipeline (a halt idles the PE >3.4us and the HAM
            # clock-gate re-throttles it to half rate).
            TRAIL = 6

            def norm_block(acc, r0, i, t2q, t2q_done):
                # Release the single acc PSUM buffer ASAP: two fast bulk
                # copies stage the denominators (f32) and the unnormalized
                # O^T rows (f16) to SBUF; the reciprocal/broadcast/multiply
                # then run off-critical-path on the SBUF copies. (The custom
                # DVE reciprocal can't read PSUM at partition 64, so den is
                # staged anyway.)
                den_raw = normp.tile([1, 2, QT], F32, name="den_raw",
                                     tag="denr")
                nc.vector.tensor_copy(out=den_raw, in_=acc[HD:HD + 1, :, :])
                o16 = normp.tile([HD, 2, QT], F16, name="o16", tag="o16")
                nc.vector.tensor_copy(out=o16, in_=acc[0:HD, :, :])
                for u in range(2):
                    den = normp.tile([1, QT], F32, name="den", tag="den")
                    nc.vector.reciprocal_approx_fast(out=den,
                                                     in_=den_raw[:, u, :])
                    bca = normp.tile([HD, QT], F32, name="bca", tag="bca")
                    nc.gpsimd.partition_broadcast(bca, den)
                    t = 2 * t2q + u
                    nc.vector.tensor_mul(
                        out=attnT_s[r0:r0 + HD, i, t * QT:(t + 1) * QT],
                        in0=o16[:, u, :], in1=bca)
                if t2q_done:
                    # this tile-pair's attnT is complete -> its output
                    # projection becomes legal filler work
                    for qc in range(2 * QT // 128):
                        filler.append(gen_outproj(t2q, qc))

            def pv_of(k, pt, acc, h, r0, i, t2q):
                force(("v", k))
                vsl = v_s[:, k, h, :]
                for u in range(2):
                    nc.tensor.matmul(
                        acc[:, u, :], vsl, pt[:, u, :],
                        start=(k == 0), stop=(k == NKC - 1),
                        skip_group_check=True)

            pend = collections.deque()

            def flush_one():
                k, pt, acc, h, r0, i, t2q = pend.popleft()
                pv_of(k, pt, acc, h, r0, i, t2q)
                if k == NKC - 1:
                    norm_block(acc, r0, i, t2q,
                               t2q_done=(i == 1 and r0 == HD))

            # blocks ordered i-outer: head-pair 1's projections aren't
            # needed until iteration 64, spreading filler work into the
            # later blocks' PE slack
            rates = [3, 3, 2, 2, 2, 1, 1, 1]
            for i in range(2):
                for t2q in range(2):
                    for hl in range(2):
                        rate = rates[i * 4 + t2q * 2 + hl]
                        h = i * 2 + hl
                        r0 = hl * HD
                        force(("q", i, 2 * t2q))
                        force(("q", i, 2 * t2q + 1))
                        acc = ps_acc.tile([HD + 1, 2, QT], F32, name="acc",
                                          tag="acc")
                        qsl = [qT_s[r0:r0 + HD, i,
                                    (2 * t2q + u) * QT:(2 * t2q + u + 1) * QT]
                               for u in range(2)]
                        for k in range(NKC):
                            if k % 4 == 0:
                                force(("k", i, k // 4))
                            st = ps_st.tile([128, 2, QT], F32, name="st",
                                            tag="st")
                            ksl = kT_s[r0:r0 + HD, i, k * KC:(k + 1) * KC]
                            for u in range(2):
                                nc.tensor.matmul(st[:, u, :], ksl, qsl[u],
                                                 start=True, stop=True)
                            pt = ptp.tile([128, 2, QT], F16, name="pt",
                                          tag="pt")
                            nc.scalar.activation(
                                out=pt.rearrange("p a b -> p (a b)"),
                                in_=st.rearrange("p a b -> p (a b)"),
                                func=mybir.ActivationFunctionType.Exp,
                                bias=ebias_s, scale=SCALE)
                            drain(rate)
                            pend.append((k, pt, acc, h, r0, i, t2q))
                            if len(pend) > TRAIL:
                                flush_one()
            while pend:
                flush_one()
                drain(2)
            drain(10 ** 9)
            if dbg:
                nc.sync.dma_start(out=dbg_qT[:, :, :], in_=qT_s)
                nc.sync.dma_start(out=dbg_kT[:, :, :], in_=kT_s)
                nc.sync.dma_start(out=dbg_v[:, :, :, :], in_=v_s)
                nc.sync.dma_start(out=dbg_attnT[:, :, :], in_=attnT_s)
    nc.finalize()
    return nc


_NC_CACHE = None


def _get_nc():
    global _NC_CACHE
    if _NC_CACHE is None:
        _NC_CACHE = build_nc()
    return _NC_CACHE


def _rope_tables():
    inv_freq = 1.0 / (10000.0 ** (np.arange(0, HD, 2, dtype=np.float32) / HD))
    t = np.arange(N, dtype=np.float32)
    freqs = np.outer(t, inv_freq).astype(np.float32)       # [N, 32]
    emb = np.concatenate([freqs, freqs], axis=-1)          # [N, 64]
    cos = np.cos(emb).astype(np.float32)                   # [N, 64]
    sin = np.sin(emb).astype(np.float32)
    idx = np.arange(128) % HD
    cosT = np.ascontiguousarray(cos.T[idx])                # [128, N]
    sgn = np.where(np.arange(HD) < HD // 2, -1.0, 1.0).astype(np.float32)
    msinT = np.ascontiguousarray((sin.T * sgn[:, None])[idx])
    msinP = np.ascontiguousarray(msinT[np.arange(128) ^ 32])
    return cosT.astype(NP16), msinP.astype(NP16)


def _pack_w(wT):
    """[n*128, C] row-major -> [128, n*C] with [p, chunk, c] free layout."""
    n = wT.shape[0] // 128
    return np.ascontiguousarray(
        wT.reshape(n, 128, -1).transpose(1, 0, 2).reshape(128, -1)
    ).astype(NP16)


def kernel(x, attention_mask, Wq, Wk, Wv, Wo):
    x = np.asarray(x, dtype=np.float32)
    Wq = np.asarray(Wq, dtype=np.float32)
    Wk = np.asarray(Wk, dtype=np.float32)
    Wv = np.asarray(Wv, dtype=np.float32)
    Wo = np.asarray(Wo, dtype=np.float32)

    cosT, msinP = _rope_tables()
    xTb = [np.ascontiguousarray(x[b].T).astype(NP16) for b in range(B)]

    in_maps = []
    for c in range(NCORES):
        b = c // (NCORES // B)
        hg = c % (NCORES // B)
        rows = slice(hg * DPC, (hg + 1) * DPC)
        in_maps.append({
            "xT": xTb[b],
            "wqd": _pack_w(Wq[rows].T),
            "wkd": _pack_w(Wk[rows].T),
            "wvd": _pack_w(Wv[rows].T),
            "wod": _pack_w(Wo[:, rows].T),
            "cosd": cosT,
            "msinPd": msinP,
        })

    global _last_in_maps
    _last_in_maps = in_maps

    nc = _get_nc()
    res = run_bass_kernel_spmd(nc, in_maps, core_ids=list(range(NCORES)))
    global _LAST_RES
    _LAST_RES = res
    parts = [r["out"].astype(np.float32) for r in res.results]

    out = np.empty((B, N, D), dtype=np.float32)
    g = NCORES // B
    for b in range(B):
        out[b] = np.sum(np.stack(parts[b * g:(b + 1) * g]), axis=0)
    return out
